# revision 19
# baseline (speedup 1.0000x reference)
"""CodeGen attention on 8 Trainium2 NeuronCores (Bass/Tile).

Sharding: tensor-parallel over the 4 CodeGen mp head-groups x data-parallel
over batch 2. Core c = dp*4 + tp handles batch dp, head group tp (4 heads).

v2 vs v1 (1.56 ms): phase-1/2 GEMM I/O in bf16 (halves the x/W streams),
X/W_qkv host-pre-tiled so every DMA line is >= 8 KB (v1's 256-512 B lines
made HW DMA descriptor-bound), Q^T/K^T/V staged in SBUF (12 MB bf16, zero
phase-2 DMA) instead of a DRAM roundtrip, and phase 3 recast as
y^T = W_out^T-slice @ og with og the *moving* operand so og streams from
DRAM exactly once with 1 KB lines and ~0.5 MB SBUF instead of 6 MB of
256 B-line gathers.

Per-core pipeline:
  phase 1: QKV projection in bf16. Q^T/K^T produced transposed (W
           stationary, X^T moving) directly into SBUF staging tiles with
           fused rotary; the rotary block of each tile group is deferred
           past the next group's matmuls so the in-order PE never waits.
           V in natural [s, d] layout into SBUF staging.
  phase 2: causal attention per head straight out of the staging tiles
           (fp32 logits/accumulation, bf16 weights and V). Inner loop
           software-pipelined two deep; per-block normalize deferred into
           the next block.
  Per-head chunked AllGather (bf16) within the 4-core batch group while
           later heads still compute.
  phase 3: out-projection y^T[c,s] accumulated over the 32 gathered
           og row-tiles (consumed in gather-arrival order), W_out slice
           stationary, og moving.

Host assembles the [B, S, D] output from per-core y^T [D/4, S] shards.
"""

import numpy as np

B, S, D = 2, 2048, 4096
N_HEAD = 16
HD = 256
MP = 4
ROT = 64
LOCAL = D // MP            # 1024 (= 4 heads * 256)
QK_TILES = 2 * LOCAL // 128  # 16: q col-tiles then k col-tiles
DT = D // 128              # 32 contraction tiles
N_CORES = 8
H_LOC = N_HEAD // MP       # 4 heads per core

_CACHE = {}


def _emit_body(nc, tc, tens, psp, cpool, with_collective, rep, phases=(1, 2, 3)):
    """One full pipeline pass (phases 1-3). rep only namespaces DRAM scratch."""
    import concourse.tile as tile  # noqa: F401
    from concourse import mybir

    f32 = mybir.dt.float32
    bf16 = mybir.dt.bfloat16
    EXP = mybir.ActivationFunctionType.Exp

    (xt_r, wqk_r, wv_r, wo_r, cost, sint, masks, y,
     og_in, og_out, og_in3, og_out3, rt_sb, ones_sb, onesr_sb) = tens

    from contextlib import ExitStack
    COPY = mybir.ActivationFunctionType.Copy

    with ExitStack() as big:
        # Persistent SBUF staging for Q^T/K^T (d-major, [128, S] per d-tile)
        # and V (s-major, [128, LOCAL] per s-tile). 12 MB bf16; written by
        # phase 1's PSUM copy-outs, consumed DMA-free by phase 2.
        stg = big.enter_context(tc.tile_pool(name="stg", bufs=1))
        qTs = [stg.tile([128, S], bf16, name=f"qT{m}") for m in range(8)]
        kTs = [stg.tile([128, S], bf16, name=f"kT{m}") for m in range(8)]
        vs = [stg.tile([128, LOCAL], bf16, name=f"v{st}") for st in range(16)]

        # ---------------- phase 1: QKV ----------------
        if 1 in phases:
         with tc.tile_pool(name="wmp", bufs=2) as wmp, \
             tc.tile_pool(name="t1p", bufs=1) as t1p, \
             tc.tile_pool(name="t2p", bufs=1) as t2p, \
             tc.tile_pool(name="wvp", bufs=2) as wvp, \
             tc.tile_pool(name="trig", bufs=1) as trig, \
             tc.tile_pool(name="xrp", bufs=1) as xrp:
            cost_sb = trig.tile([ROT, S], bf16, name="cost_sb")
            nc.sync.dma_start(cost_sb[:], cost.ap())
            sint_sb = trig.tile([ROT, S], bf16, name="sint_sb")
            nc.sync.dma_start(sint_sb[:], sint.ap())

            def qk_section(ss, xr, wm0=None):
                # Q^T / K^T (W stationary, X^T moving), rotary fused. The
                # rotary block of group g (a PE matmul gated on a DVE copy)
                # is deferred until group g+1's accumulation is underway,
                # so the in-order PE never waits on the copy.
                rot_tail = [None]
                for m in range(QK_TILES):
                    if m == 0 and wm0 is not None:
                        wm = wm0
                    else:
                        wm = wmp.tile([128, DT, 128], bf16, name="wm",
                                      tag="wm")
                        nc.sync.dma_start(wm[:], wqk_r[:, m, :, :])
                    dst = qTs[m] if m < 8 else kTs[m - 8]
                    for n in range(2):
                        ps = psp.tile([128, 512], f32, name="ps", tag="ps")
                        for dt in range(DT):
                            nc.tensor.matmul(
                                ps[:], wm[:, dt, :],
                                xr[:, dt, n * 512:(n + 1) * 512],
                                start=(dt == 0), stop=(dt == DT - 1))
                            if dt == 2 and rot_tail[0] is not None:
                                rot_tail[0]()
                                rot_tail[0] = None
                        sg = ss * 1024 + n * 512
                        sq = dst[:, sg:sg + 512]
                        with nc.allow_low_precision(reason="qk bf16"):
                            if m % 2 == 0:
                                nc.vector.tensor_copy(sq, ps[:])

                                def _rot(ps=ps, sq=sq, sg=sg):
                                    # rows 0:64 are the rotary dims of a head
                                    rp = psp.tile([128, 512], f32, name="rp",
                                                  tag="ps")
                                    nc.tensor.matmul(rp[0:ROT, :], rt_sb[:],
                                                     sq[0:ROT, :],
                                                     start=True, stop=True)
                                    t1 = t1p.tile([ROT, 512], f32, name="t1",
                                                  tag="t1")
                                    nc.vector.tensor_mul(
                                        t1[:], ps[0:ROT, :],
                                        cost_sb[:, sg:sg + 512])
                                    t2 = t2p.tile([ROT, 512], f32, name="t2",
                                                  tag="t2")
                                    nc.vector.tensor_mul(
                                        t2[:], rp[0:ROT, :],
                                        sint_sb[:, sg:sg + 512])
                                    with nc.allow_low_precision(
                                            reason="qk bf16"):
                                        nc.vector.tensor_add(
                                            sq[0:ROT, :], t1[:], t2[:])
                                rot_tail[0] = _rot
                            else:
                                # no rotary on odd tiles: ACT copy keeps the
                                # DVE free for the rotary muls
                                nc.scalar.activation(sq, ps[:], COPY,
                                                     bias=0.0, scale=1.0)
                if rot_tail[0] is not None:
                    rot_tail[0]()
                    rot_tail[0] = None

            def v_section(ss, xr):
                # V natural layout (X^T stationary, Wv moving)
                for vn in range(2):
                    pss = [psp.tile([128, 512], f32, name=f"vps{sm}",
                                    tag="ps") for sm in range(8)]
                    for dc in range(4):
                        wvc = wvp.tile([128, 8, 512], bf16, name="wvc",
                                       tag="wvc")
                        nc.sync.dma_start(
                            wvc[:], wv_r[:, vn, dc * 8:(dc + 1) * 8, :])
                        for d8 in range(8):
                            dt = dc * 8 + d8
                            for sm in range(8):
                                nc.tensor.matmul(
                                    pss[sm][:],
                                    xr[:, dt, sm * 128:(sm + 1) * 128],
                                    wvc[:, d8, :],
                                    start=(dt == 0), stop=(dt == DT - 1))
                    for sm in range(8):
                        vc = vs[ss * 8 + sm][:, vn * 512:(vn + 1) * 512]
                        with nc.allow_low_precision(reason="v bf16"):
                            # alternate copy engines so the copy-out chain
                            # at a section boundary is ~2x shorter
                            if sm % 2 == 0:
                                nc.vector.tensor_copy(vc, pss[sm][:])
                            else:
                                nc.scalar.activation(vc, pss[sm][:], COPY,
                                                     bias=0.0, scale=1.0)

            def load_xr(ss, xr=None):
                if xr is None:
                    xr = xrp.tile([128, DT, 1024], bf16, name="xr", tag="xr")
                for dc in range(4):
                    nc.sync.dma_start(
                        xr[:, dc * 8:(dc + 1) * 8, :],
                        xt_r[:, ss, dc * 8:(dc + 1) * 8, :])
                return xr

            # first wq tile (in dt chunks) ahead of the x stream so the PE
            # starts as soon as the first x dt-slices land
            wm0 = wmp.tile([128, DT, 128], bf16, name="wm", tag="wm")
            for dc in range(4):
                nc.sync.dma_start(wm0[:, dc * 8:(dc + 1) * 8, :],
                                  wqk_r[:, 0, dc * 8:(dc + 1) * 8, :])
            xr = load_xr(0)
            qk_section(0, xr, wm0)
            v_section(0, xr)
            xr = load_xr(1, xr)
            qk_section(1, xr)
            v_section(1, xr)

        # ---------------- phases 2+3: attention + out projection --------
        with ExitStack() as st:
            wo_sb = None
            if 3 in phases:
                # wop opened before the phase-2 pools (address layering);
                # its DMA stream is emitted after phase 2 starts so the 8MB
                # W_out load hides under phase-2 compute.
                wop = st.enter_context(tc.tile_pool(name="wop", bufs=1))
                wo_sb = wop.tile([128, DT, LOCAL], bf16, name="wo_sb")

            def _emit_wo_loads():
                if wo_sb is None:
                    return
                for t in range(DT):
                    wd = (t % 8) // 2 * 8 + (t // 8) * 2 + t % 2
                    nc.sync.dma_start(wo_sb[:, wd, :], wo_r[:, wd, :])

            if 2 in phases:
             with tc.tile_pool(name="mkp", bufs=1) as mkp, \
                 tc.tile_pool(name="etp", bufs=6) as etp, \
                 tc.tile_pool(name="etmp", bufs=3) as etmp, \
                 tc.tile_pool(name="otp", bufs=2) as otp, \
                 tc.tile_pool(name="rbp", bufs=2) as rbp, \
                 tc.tile_pool(name="rip", bufs=2) as rip:
                masks_sb = mkp.tile([128, 4, 512], f32, name="masks_sb")
                nc.sync.dma_start(masks_sb[:], masks.ap())
                _emit_wo_loads()

                for h in range(H_LOC):
                    qt = (qTs[2 * h], qTs[2 * h + 1])
                    kt = (kTs[2 * h], kTs[2 * h + 1])
                    ot = otp.tile([128, 2, S], bf16, name="ot", tag="ot")

                    # Deferred normalize: the reciprocal/broadcast/scale
                    # chain of block qn is emitted after block qn+1's first
                    # score matmuls, so the PE never waits on it.
                    tail = [None]

                    def _make_tail(rs, ov, q0, ot=ot):
                        def _tail():
                            rinv = rip.tile([1, 512], f32, name="rinv",
                                            tag="rinv")
                            nc.vector.reciprocal(rinv[:], rs[:])
                            rb_sb = rbp.tile([128, 512], f32, name="rb_sb",
                                             tag="rb")
                            nc.gpsimd.partition_broadcast(rb_sb[:], rinv[:])
                            with nc.allow_low_precision(reason="out bf16"):
                                for dm in range(2):
                                    nc.vector.tensor_mul(
                                        ot[:, dm, q0:q0 + 512],
                                        ov[dm][:], rb_sb[:])
                        return _tail

                    for qn in range(4):
                        nk = (qn + 1) * 4
                        q0 = qn * 512
                        rs = psp.tile([1, 512], f32, name="rs", tag="ps")
                        ov = [psp.tile([128, 512], f32, name=f"ov{dm}",
                                       tag="ps") for dm in range(2)]

                        # Software pipeline (depth 3): rowsum/PV of step ki
                        # emitted after the scores of ki+3, hiding the
                        # exp->mask round trip from the in-order PE (HW
                        # latency ~1.3us vs the ~0.9us two steps buy).
                        pending = []

                        def _flush_one(h=h, nk=nk, ov=ov, rs=rs):
                            ki0, et0 = pending.pop(0)
                            nc.tensor.matmul(rs[:], ones_sb[:], et0[:],
                                             start=(ki0 == 0),
                                             stop=(ki0 == nk - 1))
                            for dm in range(2):
                                nc.tensor.matmul(
                                    ov[dm][:],
                                    vs[ki0][:, h * HD + dm * 128:
                                            h * HD + (dm + 1) * 128],
                                    et0[:], start=(ki0 == 0),
                                    stop=(ki0 == nk - 1))

                        for ki in range(nk):
                            sp = psp.tile([128, 512], f32, name="sp",
                                          tag="ps")
                            for dd in range(2):
                                nc.tensor.matmul(
                                    sp[:],
                                    kt[dd][:, ki * 128:(ki + 1) * 128],
                                    qt[dd][:, q0:q0 + 512],
                                    start=(dd == 0), stop=(dd == 1))
                            et = etp.tile([128, 512], bf16, name="et",
                                          tag="et")
                            with nc.allow_low_precision(reason="wts bf16"):
                                if ki >= qn * 4:
                                    etm = etmp.tile([128, 512], f32,
                                                    name="etm", tag="etm")
                                    nc.scalar.activation(etm[:], sp[:], EXP,
                                                         bias=0.0,
                                                         scale=1.0 / 16.0)
                                    nc.vector.tensor_mul(
                                        et[:], etm[:],
                                        masks_sb[:, ki - qn * 4, :])
                                else:
                                    nc.scalar.activation(et[:], sp[:], EXP,
                                                         bias=0.0,
                                                         scale=1.0 / 16.0)
                            pending.append((ki, et))
                            if ki == 1 and tail[0] is not None:
                                tail[0]()
                                tail[0] = None
                            if len(pending) > 3:
                                _flush_one()
                        while pending:
                            _flush_one()
                        tail[0] = _make_tail(rs, ov, q0)
                    tail[0]()
                    tail[0] = None

                    if h < H_LOC - 1 or 3 not in phases:
                        for dm in range(2):
                            rr = (h * 2 + dm) * 128
                            nc.sync.dma_start(og_in[rr:rr + 128, :],
                                              ot[:, dm, :])
                    else:
                        # head 3 lands in s-quarter-blocked scratch so its
                        # gather can be split into 4 contiguous collectives
                        for sq in range(4):
                            for dm in range(2):
                                nc.sync.dma_start(
                                    og_in3[sq * 256 + dm * 128:
                                           sq * 256 + (dm + 1) * 128, :],
                                    ot[:, dm, sq * 512:(sq + 1) * 512])

                    # Per-head chunked AllGather: head h's 256-row stripe is
                    # gathered while heads h+1.. still compute. Chunk h of
                    # og_out holds [rank0 h-stripe; rank1 h-stripe; ...].
                    # The last head can't hide behind later compute, so its
                    # gather is further split into the 4 s-quarters phase 3
                    # consumes (collectives need contiguous patterns, hence
                    # the dedicated quarter-blocked scratch), pipelining it
                    # against phase-3 rounds.
                    if 3 in phases:
                        if h < H_LOC - 1:
                            chunks = [(og_in[h * 256:(h + 1) * 256, :],
                                       og_out[h * LOCAL:(h + 1) * LOCAL, :],
                                       S)]
                        else:
                            chunks = [
                                (og_in3[sq * 256:(sq + 1) * 256, :],
                                 og_out3[sq * 1024:(sq + 1) * 1024, :],
                                 512)
                                for sq in range(4)]
                        for cin, cout, w in chunks:
                            if with_collective:
                                nc.gpsimd.collective_compute(
                                    "AllGather",
                                    mybir.AluOpType.bypass,
                                    replica_groups=[[0, 1, 2, 3],
                                                    [4, 5, 6, 7]],
                                    ins=[cin.opt()],
                                    outs=[cout.opt()],
                                )
                            else:
                                for blk in range(MP):
                                    nc.sync.dma_start(
                                        cout[blk * 256:(blk + 1) * 256, :],
                                        cin)

            if 3 not in phases:
                return

            # ---------------- phase 3: out projection ----------------
            # y^T[c, s] = sum_d W_out[d, c] og[d, s]; og row-tile t of the
            # chunked og_out (t = h*8 + rank*2 + dhalf) pairs with W_out
            # row-tile wd = rank*8 + h*2 + dhalf. og is the moving operand,
            # streamed from DRAM once per s-block with 1 KB lines.
            og_ch = [og_out[hh * LOCAL:(hh + 1) * LOCAL, :]
                     .rearrange("(dt p) s -> p dt s", p=128)
                     for hh in range(MP - 1)]
            with tc.tile_pool(name="omp", bufs=6) as omp, \
                 tc.tile_pool(name="resp", bufs=4) as resp:
                for sb in range(4):
                    s0 = sb * 512
                    ps3 = [psp.tile([128, 512], f32, name=f"ps3_{cb}",
                                    tag="ps") for cb in range(8)]
                    for t in range(DT):
                        hh, sub = t // 8, t % 8
                        wd = (sub // 2) * 8 + hh * 2 + (sub % 2)
                        ogm = omp.tile([128, 512], bf16, name="ogm",
                                       tag="om")
                        if hh < MP - 1:
                            nc.sync.dma_start(
                                ogm[:], og_ch[hh][:, sub, s0:s0 + 512])
                        else:
                            nc.sync.dma_start(
                                ogm[:],
                                og_out3[sb * 1024 + sub * 128:
                                        sb * 1024 + (sub + 1) * 128, :])
                        for cb in range(8):
                            nc.tensor.matmul(
                                ps3[cb][:],
                                wo_sb[:, wd, cb * 128:(cb + 1) * 128],
                                ogm[:], start=(t == 0), stop=(t == DT - 1))
                    for cb in range(8):
                        res = resp.tile([128, 512], f32, name="res",
                                        tag="res")
                        if cb % 2 == 0:
                            nc.vector.tensor_copy(res[:], ps3[cb][:])
                        else:
                            nc.scalar.activation(res[:], ps3[cb][:],
                                                 COPY, bias=0.0, scale=1.0)
                        nc.sync.dma_start(
                            y.ap()[cb * 128:(cb + 1) * 128, s0:s0 + 512],
                            res[:])


def _build_program(with_collective=True, n_repeat=1, phases=(1, 2, 3)):
    import concourse.bass as bass  # noqa: F401
    import concourse.tile as tile
    from concourse import bacc, mybir

    f32 = mybir.dt.float32
    bf16 = mybir.dt.bfloat16

    nc = bacc.Bacc("TRN2", target_bir_lowering=False, debug=False,
                   enable_asserts=True, num_devices=N_CORES)

    # Pre-tiled inputs (host does the layout): every DMA line >= 8 KB.
    #   xt:  [128, 2(ss), 32(dt), 1024(s)] -> per-partition 64 KB contig
    #   wqk: [128, 16(m), 32(dt), 128(c)]  -> per-(p,m) 8 KB contig
    #   wv:  [128, 2(vn), 32(dt), 512(c)]  -> per-(p,vn,8dt) 8 KB chunks
    xt = nc.dram_tensor("xt", [128, 2 * DT * 1024], bf16,
                        kind="ExternalInput")
    wqk = nc.dram_tensor("wqk", [128, QK_TILES * DT * 128], bf16,
                         kind="ExternalInput")
    wv = nc.dram_tensor("wv", [128, 2 * DT * 512], bf16,
                        kind="ExternalInput")
    wo = nc.dram_tensor("wo", [D, LOCAL], bf16, kind="ExternalInput")
    cost = nc.dram_tensor("cost", [ROT, S], bf16, kind="ExternalInput")
    sint = nc.dram_tensor("sint", [ROT, S], bf16, kind="ExternalInput")
    rt = nc.dram_tensor("rt", [ROT, ROT], bf16, kind="ExternalInput")
    ones = nc.dram_tensor("ones", [128, 1], bf16, kind="ExternalInput")
    onesr = nc.dram_tensor("onesr", [1, 128], bf16, kind="ExternalInput")
    masks = nc.dram_tensor("masks", [128, 4, 512], f32, kind="ExternalInput")
    y = nc.dram_tensor("y", [LOCAL, S], f32, kind="ExternalOutput")

    xt_r = xt.ap().rearrange("p (ss dt s) -> p ss dt s", ss=2, dt=DT)
    wqk_r = wqk.ap().rearrange("p (m dt c) -> p m dt c", m=QK_TILES, dt=DT)
    wv_r = wv.ap().rearrange("p (vn dt c) -> p vn dt c", vn=2, dt=DT)
    wo_r = wo.ap().rearrange("(dt p) c -> p dt c", p=128)

    with tile.TileContext(nc) as tc:
        with tc.tile_pool(name="dram", bufs=1, space="DRAM") as dpool, \
             tc.tile_pool(name="const", bufs=1) as cpool, \
             tc.tile_pool(name="psum", bufs=8, space="PSUM") as psp:
            og_in = dpool.tile([LOCAL, S], bf16, name="og_in")
            og_out = dpool.tile([MP * LOCAL, S], bf16, name="og_out")
            og_in3 = dpool.tile([4 * 256, 512], bf16, name="og_in3")
            og_out3 = dpool.tile([4 * 1024, 512], bf16, name="og_out3")

            rt_sb = cpool.tile([ROT, ROT], bf16, name="rt_sb")
            nc.sync.dma_start(rt_sb[:], rt.ap())
            ones_sb = cpool.tile([128, 1], bf16, name="ones_sb")
            nc.sync.dma_start(ones_sb[:], ones.ap())
            onesr_sb = cpool.tile([1, 128], bf16, name="onesr_sb")
            nc.sync.dma_start(onesr_sb[:], onesr.ap())

            tens = (xt_r, wqk_r, wv_r, wo_r, cost, sint, masks, y,
                    og_in, og_out, og_in3, og_out3, rt_sb, ones_sb,
                    onesr_sb)
            for rep in range(n_repeat):
                _emit_body(nc, tc, tens, psp, cpool, with_collective, rep,
                           phases=phases)

    nc.compile()
    return nc


def _rotary_tables(position_ids):
    """Transposed, interleave-repeated sin/cos tables: [64, S] per batch."""
    import ml_dtypes
    pos = np.asarray(position_ids).astype(np.int64)
    inv_freq = 1.0 / (10000.0 ** (np.arange(0, ROT, 2, dtype=np.float32) / ROT))
    sinusoid = np.arange(2048, dtype=np.float32)[:, None] * inv_freq[None, :]
    sin_t = np.sin(sinusoid).astype(np.float32)   # [2048, 32]
    cos_t = np.cos(sinusoid).astype(np.float32)
    outs = []
    for b in range(pos.shape[0]):
        sg = np.repeat(sin_t[pos[b]], 2, axis=1).T   # [64, S]
        cg = np.repeat(cos_t[pos[b]], 2, axis=1).T
        outs.append((np.ascontiguousarray(sg).astype(ml_dtypes.bfloat16),
                     np.ascontiguousarray(cg).astype(ml_dtypes.bfloat16)))
    return outs


def _consts():
    import ml_dtypes
    rt_np = np.zeros((ROT, ROT), dtype=np.float32)
    for i in range(ROT // 2):
        rt_np[2 * i + 1, 2 * i] = -1.0   # rt = R^T for rotate_every_two
        rt_np[2 * i, 2 * i + 1] = 1.0
    rt_np = rt_np.astype(ml_dtypes.bfloat16)
    ones_np = np.ones((128, 1), dtype=ml_dtypes.bfloat16)
    onesr_np = np.ones((1, 128), dtype=ml_dtypes.bfloat16)
    masks_np = np.zeros((128, 4, 512), dtype=np.float32)
    ii = np.arange(128)[:, None]
    qq = np.arange(512)[None, :]
    for j in range(4):
        masks_np[:, j, :] = (128 * j + ii <= qq).astype(np.float32)
    return rt_np, ones_np, onesr_np, masks_np


def _tile_x(xt):
    # [4096, 2048] -> [128, 2(ss) * 32(dt) * 1024(s)]
    return np.ascontiguousarray(
        xt.reshape(DT, 128, 2, 1024).transpose(1, 2, 0, 3).reshape(128, -1))


def _tile_wqk(w):
    # [4096, 1024] -> [128, 8(m) * 32(dt) * 128(c)]
    return np.ascontiguousarray(
        w.reshape(DT, 128, 8, 128).transpose(1, 2, 0, 3).reshape(128, -1))


def _tile_wv(w):
    # [4096, 1024] -> [128, 2(vn) * 32(dt) * 512(c)]
    return np.ascontiguousarray(
        w.reshape(DT, 128, 2, 512).transpose(1, 2, 0, 3).reshape(128, -1))


def _in_maps(hidden_states, position_ids, W_qkv, W_out):
    import ml_dtypes
    bf16 = ml_dtypes.bfloat16
    hs = np.asarray(hidden_states, dtype=np.float32)
    wqkv = np.asarray(W_qkv, dtype=np.float32)
    wout = np.asarray(W_out, dtype=np.float32)
    rt_np, ones_np, onesr_np, masks_np = _consts()
    trig = _rotary_tables(position_ids)

    xts = [_tile_x(np.ascontiguousarray(hs[b].T).astype(bf16))
           for b in range(B)]
    in_maps = []
    for c in range(N_CORES):
        dp, tp = c // MP, c % MP
        wl = wqkv[:, tp * 3 * LOCAL:(tp + 1) * 3 * LOCAL].astype(bf16)
        sg, cg = trig[dp]
        wq_t = _tile_wqk(wl[:, 0:LOCAL])
        wk_t = _tile_wqk(wl[:, 2 * LOCAL:3 * LOCAL])
        in_maps.append({
            "xt": xts[dp],
            "wqk": np.concatenate([wq_t, wk_t], axis=1),
            "wv": _tile_wv(wl[:, LOCAL:2 * LOCAL]),
            "wo": np.ascontiguousarray(
                wout[:, tp * LOCAL:(tp + 1) * LOCAL]).astype(bf16),
            "cost": cg, "sint": sg,
            "rt": rt_np, "ones": ones_np, "onesr": onesr_np,
            "masks": masks_np,
        })
    return in_maps


def _get_runner(n_repeat=1, phases=(1, 2, 3), with_collective=True):
    key = ("runner", n_repeat, tuple(phases), with_collective)
    if key in _CACHE:
        return _CACHE[key]
    import jax
    from jax.sharding import Mesh, PartitionSpec, NamedSharding
    from jax.experimental.shard_map import shard_map
    from concourse import bass2jax, mybir

    nc = _build_program(with_collective=with_collective, n_repeat=n_repeat,
                        phases=phases)
    bass2jax.install_neuronx_cc_hook()

    partition_name = (nc.partition_id_tensor.name
                      if nc.partition_id_tensor else None)
    in_names, out_names, out_avals, zero_outs = [], [], [], []
    for alloc in nc.m.functions[0].allocations:
        if not isinstance(alloc, mybir.MemoryLocationSet):
            continue
        name = alloc.memorylocations[0].name
        if alloc.kind == "ExternalInput":
            if name != partition_name:
                in_names.append(name)
        elif alloc.kind == "ExternalOutput":
            shape = tuple(alloc.tensor_shape)
            dtype = mybir.dt.np(alloc.dtype)
            out_names.append(name)
            out_avals.append(jax.core.ShapedArray(shape, dtype))
            zero_outs.append(np.zeros(shape, dtype))
    n_params = len(in_names)
    all_names = in_names + out_names
    if partition_name is not None:
        all_names = all_names + [partition_name]

    def _body(*args):
        operands = list(args)
        if partition_name is not None:
            operands.append(bass2jax.partition_id_tensor())
        outs = bass2jax._bass_exec_p.bind(
            *operands,
            out_avals=tuple(out_avals),
            in_names=tuple(all_names),
            out_names=tuple(out_names),
            lowering_input_output_aliases=(),
            sim_require_finite=True,
            sim_require_nnan=True,
            nc=nc,
        )
        return tuple(outs)

    devices = jax.devices()[:N_CORES]
    mesh = Mesh(np.asarray(devices), ("core",))
    n_outs = len(out_names)
    sharded = jax.jit(
        shard_map(_body, mesh=mesh,
                  in_specs=(PartitionSpec("core"),) * (n_params + n_outs),
                  out_specs=(PartitionSpec("core"),) * n_outs,
                  check_rep=False),
        keep_unused=True,
    )
    sharding = NamedSharding(mesh, PartitionSpec("core"))
    runner = {
        "nc": nc, "sharded": sharded, "in_names": in_names,
        "out_names": out_names, "out_avals": out_avals,
        "zero_outs": zero_outs, "sharding": sharding, "jax": jax,
    }
    _CACHE[key] = runner
    return runner


def _stage(runner, in_maps):
    jax = runner["jax"]
    concat_in = [
        np.concatenate([np.asarray(in_maps[c][name]) for c in range(N_CORES)],
                       axis=0)
        for name in runner["in_names"]
    ]
    concat_zero = [
        np.zeros((N_CORES * z.shape[0], *z.shape[1:]), z.dtype)
        for z in runner["zero_outs"]
    ]
    return [jax.device_put(a, runner["sharding"]) for a in concat_in + concat_zero]


def _execute(runner, staged):
    jax = runner["jax"]
    outs = runner["sharded"](*staged)
    outs = jax.block_until_ready(outs)
    return outs


def kernel(hidden_states, position_ids, W_qkv, W_out):
    runner = _get_runner()
    in_maps = _in_maps(hidden_states, position_ids, W_qkv, W_out)
    staged = _stage(runner, in_maps)
    outs = _execute(runner, staged)
    yc = np.asarray(outs[0]).reshape(N_CORES, LOCAL, S)
    result = np.empty((B, S, D), dtype=np.float32)
    for c in range(N_CORES):
        dp, tp = c // MP, c % MP
        result[dp][:, tp * LOCAL:(tp + 1) * LOCAL] = yc[c].T
    return result


def bench(inputs, iters=10, n_repeat=1):
    """Return per-call wall-clock seconds (list) for the staged executable."""
    import time
    runner = _get_runner(n_repeat)
    in_maps = _in_maps(**inputs)
    staged = _stage(runner, in_maps)
    _execute(runner, staged)  # warm-up / compile
    times = []
    for _ in range(iters):
        t0 = time.perf_counter()
        _execute(runner, staged)
        times.append(time.perf_counter() - t0)
    return times


# revision 20
# speedup vs baseline: 1.0389x; 1.0389x over previous
"""CodeGen attention on 8 Trainium2 NeuronCores (Bass/Tile).

Sharding: tensor-parallel over the 4 CodeGen mp head-groups x data-parallel
over batch 2. Core c = dp*4 + tp handles batch dp, head group tp (4 heads).

v2 vs v1 (1.56 ms): phase-1/2 GEMM I/O in bf16 (halves the x/W streams),
X/W_qkv host-pre-tiled so every DMA line is >= 8 KB (v1's 256-512 B lines
made HW DMA descriptor-bound), Q^T/K^T/V staged in SBUF (12 MB bf16, zero
phase-2 DMA) instead of a DRAM roundtrip, and phase 3 recast as
y^T = W_out^T-slice @ og with og the *moving* operand so og streams from
DRAM exactly once with 1 KB lines and ~0.5 MB SBUF instead of 6 MB of
256 B-line gathers.

Per-core pipeline:
  phase 1: QKV projection in bf16. Q^T/K^T produced transposed (W
           stationary, X^T moving) directly into SBUF staging tiles with
           fused rotary; the rotary block of each tile group is deferred
           past the next group's matmuls so the in-order PE never waits.
           V in natural [s, d] layout into SBUF staging.
  phase 2: causal attention per head straight out of the staging tiles
           (fp32 logits/accumulation, bf16 weights and V). Inner loop
           software-pipelined two deep; per-block normalize deferred into
           the next block.
  Per-head chunked AllGather (bf16) within the 4-core batch group while
           later heads still compute.
  phase 3: out-projection y^T[c,s] accumulated over the 32 gathered
           og row-tiles (consumed in gather-arrival order), W_out slice
           stationary, og moving.

Host assembles the [B, S, D] output from per-core y^T [D/4, S] shards.
"""

import numpy as np

B, S, D = 2, 2048, 4096
N_HEAD = 16
HD = 256
MP = 4
ROT = 64
LOCAL = D // MP            # 1024 (= 4 heads * 256)
QK_TILES = 2 * LOCAL // 128  # 16: q col-tiles then k col-tiles
DT = D // 128              # 32 contraction tiles
N_CORES = 8
H_LOC = N_HEAD // MP       # 4 heads per core

_CACHE = {}


def _emit_body(nc, tc, tens, psp, cpool, with_collective, rep, phases=(1, 2, 3)):
    """One full pipeline pass (phases 1-3). rep only namespaces DRAM scratch."""
    import concourse.tile as tile  # noqa: F401
    from concourse import mybir

    f32 = mybir.dt.float32
    bf16 = mybir.dt.bfloat16
    EXP = mybir.ActivationFunctionType.Exp

    (xt_r, wqk_r, wv_r, wo_r, cost, sint, masks, y,
     og_in, og_out, og_in3, og_out3, rt_sb, ones_sb, onesr_sb) = tens

    from contextlib import ExitStack
    COPY = mybir.ActivationFunctionType.Copy

    with ExitStack() as big:
        # Persistent SBUF staging for Q^T/K^T (d-major, [128, S] per d-tile)
        # and V (s-major, [128, LOCAL] per s-tile). 12 MB bf16; written by
        # phase 1's PSUM copy-outs, consumed DMA-free by phase 2.
        stg = big.enter_context(tc.tile_pool(name="stg", bufs=1))
        qTs = [stg.tile([128, S], bf16, name=f"qT{m}") for m in range(8)]
        kTs = [stg.tile([128, S], bf16, name=f"kT{m}") for m in range(8)]
        vs = [stg.tile([128, LOCAL], bf16, name=f"v{st}") for st in range(16)]

        # ---------------- phase 1: QKV ----------------
        if 1 in phases:
         with tc.tile_pool(name="wmp", bufs=2) as wmp, \
             tc.tile_pool(name="t1p", bufs=1) as t1p, \
             tc.tile_pool(name="t2p", bufs=1) as t2p, \
             tc.tile_pool(name="wvp", bufs=2) as wvp, \
             tc.tile_pool(name="trig", bufs=1) as trig, \
             tc.tile_pool(name="xrp", bufs=1) as xrp:
            cost_sb = trig.tile([ROT, S], bf16, name="cost_sb")
            nc.sync.dma_start(cost_sb[:], cost.ap())
            sint_sb = trig.tile([ROT, S], bf16, name="sint_sb")
            nc.sync.dma_start(sint_sb[:], sint.ap())

            def qk_section(ss, xr, wm0=None):
                # Q^T / K^T (W stationary, X^T moving), rotary fused. The
                # rotary block of group g (a PE matmul gated on a DVE copy)
                # is deferred until group g+1's accumulation is underway,
                # so the in-order PE never waits on the copy.
                rot_tail = [None]
                for m in range(QK_TILES):
                    if m == 0 and wm0 is not None:
                        wm = wm0
                    else:
                        wm = wmp.tile([128, DT, 128], bf16, name="wm",
                                      tag="wm")
                        # weight streams ride the ACT HWDGE ring so they
                        # don't serialize with the x stream on the SP ring
                        nc.scalar.dma_start(wm[:], wqk_r[:, m, :, :])
                    dst = qTs[m] if m < 8 else kTs[m - 8]
                    for n in range(2):
                        ps = psp.tile([128, 512], f32, name="ps", tag="ps")
                        for dt in range(DT):
                            nc.tensor.matmul(
                                ps[:], wm[:, dt, :],
                                xr[:, dt, n * 512:(n + 1) * 512],
                                start=(dt == 0), stop=(dt == DT - 1))
                            if dt == 2 and rot_tail[0] is not None:
                                rot_tail[0]()
                                rot_tail[0] = None
                        sg = ss * 1024 + n * 512
                        sq = dst[:, sg:sg + 512]
                        with nc.allow_low_precision(reason="qk bf16"):
                            if m % 2 == 0:
                                nc.vector.tensor_copy(sq, ps[:])

                                def _rot(ps=ps, sq=sq, sg=sg):
                                    # rows 0:64 are the rotary dims of a head
                                    rp = psp.tile([128, 512], f32, name="rp",
                                                  tag="ps")
                                    nc.tensor.matmul(rp[0:ROT, :], rt_sb[:],
                                                     sq[0:ROT, :],
                                                     start=True, stop=True)
                                    t1 = t1p.tile([ROT, 512], f32, name="t1",
                                                  tag="t1")
                                    nc.vector.tensor_mul(
                                        t1[:], ps[0:ROT, :],
                                        cost_sb[:, sg:sg + 512])
                                    t2 = t2p.tile([ROT, 512], f32, name="t2",
                                                  tag="t2")
                                    nc.vector.tensor_mul(
                                        t2[:], rp[0:ROT, :],
                                        sint_sb[:, sg:sg + 512])
                                    with nc.allow_low_precision(
                                            reason="qk bf16"):
                                        nc.vector.tensor_add(
                                            sq[0:ROT, :], t1[:], t2[:])
                                rot_tail[0] = _rot
                            else:
                                # no rotary on odd tiles: ACT copy keeps the
                                # DVE free for the rotary muls
                                nc.scalar.activation(sq, ps[:], COPY,
                                                     bias=0.0, scale=1.0)
                if rot_tail[0] is not None:
                    rot_tail[0]()
                    rot_tail[0] = None

            def v_section(ss, xr):
                # V natural layout (X^T stationary, Wv moving)
                for vn in range(2):
                    pss = [psp.tile([128, 512], f32, name=f"vps{sm}",
                                    tag="ps") for sm in range(8)]
                    for dc in range(4):
                        wvc = wvp.tile([128, 8, 512], bf16, name="wvc",
                                       tag="wvc")
                        nc.scalar.dma_start(
                            wvc[:], wv_r[:, vn, dc * 8:(dc + 1) * 8, :])
                        for d8 in range(8):
                            dt = dc * 8 + d8
                            for sm in range(8):
                                nc.tensor.matmul(
                                    pss[sm][:],
                                    xr[:, dt, sm * 128:(sm + 1) * 128],
                                    wvc[:, d8, :],
                                    start=(dt == 0), stop=(dt == DT - 1))
                    for sm in range(8):
                        vc = vs[ss * 8 + sm][:, vn * 512:(vn + 1) * 512]
                        with nc.allow_low_precision(reason="v bf16"):
                            # alternate copy engines so the copy-out chain
                            # at a section boundary is ~2x shorter
                            if sm % 2 == 0:
                                nc.vector.tensor_copy(vc, pss[sm][:])
                            else:
                                nc.scalar.activation(vc, pss[sm][:], COPY,
                                                     bias=0.0, scale=1.0)

            def load_xr(ss, xr=None):
                if xr is None:
                    xr = xrp.tile([128, DT, 1024], bf16, name="xr", tag="xr")
                for dc in range(4):
                    nc.sync.dma_start(
                        xr[:, dc * 8:(dc + 1) * 8, :],
                        xt_r[:, ss, dc * 8:(dc + 1) * 8, :])
                return xr

            # first wq tile (in dt chunks) ahead of the x stream so the PE
            # starts as soon as the first x dt-slices land
            wm0 = wmp.tile([128, DT, 128], bf16, name="wm", tag="wm")
            for dc in range(4):
                nc.scalar.dma_start(wm0[:, dc * 8:(dc + 1) * 8, :],
                                    wqk_r[:, 0, dc * 8:(dc + 1) * 8, :])
            xr = load_xr(0)
            qk_section(0, xr, wm0)
            v_section(0, xr)
            xr = load_xr(1, xr)
            qk_section(1, xr)
            v_section(1, xr)

        # ---------------- phases 2+3: attention + out projection --------
        with ExitStack() as st:
            wo_sb = None
            if 3 in phases:
                # wop opened before the phase-2 pools (address layering);
                # its DMA stream is emitted after phase 2 starts so the 8MB
                # W_out load hides under phase-2 compute.
                wop = st.enter_context(tc.tile_pool(name="wop", bufs=1))
                wo_sb = wop.tile([128, DT, LOCAL], bf16, name="wo_sb")

            def _emit_wo_loads():
                if wo_sb is None:
                    return
                for t in range(DT):
                    wd = (t % 8) // 2 * 8 + (t // 8) * 2 + t % 2
                    nc.sync.dma_start(wo_sb[:, wd, :], wo_r[:, wd, :])

            if 2 in phases:
             with tc.tile_pool(name="mkp", bufs=1) as mkp, \
                 tc.tile_pool(name="etp", bufs=6) as etp, \
                 tc.tile_pool(name="etmp", bufs=3) as etmp, \
                 tc.tile_pool(name="otp", bufs=2) as otp, \
                 tc.tile_pool(name="rbp", bufs=2) as rbp, \
                 tc.tile_pool(name="rip", bufs=2) as rip:
                masks_sb = mkp.tile([128, 4, 512], f32, name="masks_sb")
                nc.sync.dma_start(masks_sb[:], masks.ap())
                _emit_wo_loads()

                for h in range(H_LOC):
                    qt = (qTs[2 * h], qTs[2 * h + 1])
                    kt = (kTs[2 * h], kTs[2 * h + 1])
                    ot = otp.tile([128, 2, S], bf16, name="ot", tag="ot")

                    # Deferred normalize: the reciprocal/broadcast/scale
                    # chain of block qn is emitted after block qn+1's first
                    # score matmuls, so the PE never waits on it.
                    tail = [None]

                    def _make_tail(rs, ov, q0, ot=ot):
                        def _tail():
                            rinv = rip.tile([1, 512], f32, name="rinv",
                                            tag="rinv")
                            nc.vector.reciprocal(rinv[:], rs[:])
                            rb_sb = rbp.tile([128, 512], f32, name="rb_sb",
                                             tag="rb")
                            nc.gpsimd.partition_broadcast(rb_sb[:], rinv[:])
                            with nc.allow_low_precision(reason="out bf16"):
                                for dm in range(2):
                                    nc.vector.tensor_mul(
                                        ot[:, dm, q0:q0 + 512],
                                        ov[dm][:], rb_sb[:])
                        return _tail

                    for qn in range(4):
                        nk = (qn + 1) * 4
                        q0 = qn * 512
                        rs = psp.tile([1, 512], f32, name="rs", tag="ps")
                        ov = [psp.tile([128, 512], f32, name=f"ov{dm}",
                                       tag="ps") for dm in range(2)]

                        # Software pipeline (depth 3): rowsum/PV of step ki
                        # emitted after the scores of ki+3, hiding the
                        # exp->mask round trip from the in-order PE (HW
                        # latency ~1.3us vs the ~0.9us two steps buy).
                        pending = []

                        def _flush_one(h=h, nk=nk, ov=ov, rs=rs):
                            ki0, et0 = pending.pop(0)
                            nc.tensor.matmul(rs[:], ones_sb[:], et0[:],
                                             start=(ki0 == 0),
                                             stop=(ki0 == nk - 1))
                            for dm in range(2):
                                nc.tensor.matmul(
                                    ov[dm][:],
                                    vs[ki0][:, h * HD + dm * 128:
                                            h * HD + (dm + 1) * 128],
                                    et0[:], start=(ki0 == 0),
                                    stop=(ki0 == nk - 1))

                        for ki in range(nk):
                            sp = psp.tile([128, 512], f32, name="sp",
                                          tag="ps")
                            for dd in range(2):
                                nc.tensor.matmul(
                                    sp[:],
                                    kt[dd][:, ki * 128:(ki + 1) * 128],
                                    qt[dd][:, q0:q0 + 512],
                                    start=(dd == 0), stop=(dd == 1))
                            et = etp.tile([128, 512], bf16, name="et",
                                          tag="et")
                            with nc.allow_low_precision(reason="wts bf16"):
                                if ki >= qn * 4:
                                    etm = etmp.tile([128, 512], f32,
                                                    name="etm", tag="etm")
                                    nc.scalar.activation(etm[:], sp[:], EXP,
                                                         bias=0.0,
                                                         scale=1.0 / 16.0)
                                    nc.vector.tensor_mul(
                                        et[:], etm[:],
                                        masks_sb[:, ki - qn * 4, :])
                                else:
                                    nc.scalar.activation(et[:], sp[:], EXP,
                                                         bias=0.0,
                                                         scale=1.0 / 16.0)
                            pending.append((ki, et))
                            if ki == 1 and tail[0] is not None:
                                tail[0]()
                                tail[0] = None
                            if len(pending) > 3:
                                _flush_one()
                        while pending:
                            _flush_one()
                        tail[0] = _make_tail(rs, ov, q0)
                    tail[0]()
                    tail[0] = None

                    if h < H_LOC - 1 or 3 not in phases:
                        for dm in range(2):
                            rr = (h * 2 + dm) * 128
                            nc.sync.dma_start(og_in[rr:rr + 128, :],
                                              ot[:, dm, :])
                    else:
                        # head 3 lands in s-quarter-blocked scratch so its
                        # gather can be split into 4 contiguous collectives
                        for sq in range(4):
                            for dm in range(2):
                                nc.sync.dma_start(
                                    og_in3[sq * 256 + dm * 128:
                                           sq * 256 + (dm + 1) * 128, :],
                                    ot[:, dm, sq * 512:(sq + 1) * 512])

                    # Per-head chunked AllGather: head h's 256-row stripe is
                    # gathered while heads h+1.. still compute. Chunk h of
                    # og_out holds [rank0 h-stripe; rank1 h-stripe; ...].
                    # The last head can't hide behind later compute, so its
                    # gather is further split into the 4 s-quarters phase 3
                    # consumes (collectives need contiguous patterns, hence
                    # the dedicated quarter-blocked scratch), pipelining it
                    # against phase-3 rounds.
                    if 3 in phases:
                        if h < H_LOC - 1:
                            chunks = [(og_in[h * 256:(h + 1) * 256, :],
                                       og_out[h * LOCAL:(h + 1) * LOCAL, :],
                                       S)]
                        else:
                            chunks = [
                                (og_in3[sq * 256:(sq + 1) * 256, :],
                                 og_out3[sq * 1024:(sq + 1) * 1024, :],
                                 512)
                                for sq in range(4)]
                        for cin, cout, w in chunks:
                            if with_collective:
                                nc.gpsimd.collective_compute(
                                    "AllGather",
                                    mybir.AluOpType.bypass,
                                    replica_groups=[[0, 1, 2, 3],
                                                    [4, 5, 6, 7]],
                                    ins=[cin.opt()],
                                    outs=[cout.opt()],
                                )
                            else:
                                for blk in range(MP):
                                    nc.sync.dma_start(
                                        cout[blk * 256:(blk + 1) * 256, :],
                                        cin)

            if 3 not in phases:
                return

            # ---------------- phase 3: out projection ----------------
            # y^T[c, s] = sum_d W_out[d, c] og[d, s]; og row-tile t of the
            # chunked og_out (t = h*8 + rank*2 + dhalf) pairs with W_out
            # row-tile wd = rank*8 + h*2 + dhalf. og is the moving operand,
            # streamed from DRAM once per s-block with 1 KB lines.
            og_ch = [og_out[hh * LOCAL:(hh + 1) * LOCAL, :]
                     .rearrange("(dt p) s -> p dt s", p=128)
                     for hh in range(MP - 1)]
            with tc.tile_pool(name="omp", bufs=6) as omp, \
                 tc.tile_pool(name="resp", bufs=4) as resp:
                for sb in range(4):
                    s0 = sb * 512
                    ps3 = [psp.tile([128, 512], f32, name=f"ps3_{cb}",
                                    tag="ps") for cb in range(8)]
                    for t in range(DT):
                        hh, sub = t // 8, t % 8
                        wd = (sub // 2) * 8 + hh * 2 + (sub % 2)
                        ogm = omp.tile([128, 512], bf16, name="ogm",
                                       tag="om")
                        eng = nc.sync if t % 2 == 0 else nc.scalar
                        if hh < MP - 1:
                            eng.dma_start(
                                ogm[:], og_ch[hh][:, sub, s0:s0 + 512])
                        else:
                            eng.dma_start(
                                ogm[:],
                                og_out3[sb * 1024 + sub * 128:
                                        sb * 1024 + (sub + 1) * 128, :])
                        for cb in range(8):
                            nc.tensor.matmul(
                                ps3[cb][:],
                                wo_sb[:, wd, cb * 128:(cb + 1) * 128],
                                ogm[:], start=(t == 0), stop=(t == DT - 1))
                    for cb in range(8):
                        res = resp.tile([128, 512], f32, name="res",
                                        tag="res")
                        if cb % 2 == 0:
                            nc.vector.tensor_copy(res[:], ps3[cb][:])
                        else:
                            nc.scalar.activation(res[:], ps3[cb][:],
                                                 COPY, bias=0.0, scale=1.0)
                        nc.sync.dma_start(
                            y.ap()[cb * 128:(cb + 1) * 128, s0:s0 + 512],
                            res[:])


def _build_program(with_collective=True, n_repeat=1, phases=(1, 2, 3)):
    import concourse.bass as bass  # noqa: F401
    import concourse.tile as tile
    from concourse import bacc, mybir

    f32 = mybir.dt.float32
    bf16 = mybir.dt.bfloat16

    nc = bacc.Bacc("TRN2", target_bir_lowering=False, debug=False,
                   enable_asserts=True, num_devices=N_CORES)

    # Pre-tiled inputs (host does the layout): every DMA line >= 8 KB.
    #   xt:  [128, 2(ss), 32(dt), 1024(s)] -> per-partition 64 KB contig
    #   wqk: [128, 16(m), 32(dt), 128(c)]  -> per-(p,m) 8 KB contig
    #   wv:  [128, 2(vn), 32(dt), 512(c)]  -> per-(p,vn,8dt) 8 KB chunks
    xt = nc.dram_tensor("xt", [128, 2 * DT * 1024], bf16,
                        kind="ExternalInput")
    wqk = nc.dram_tensor("wqk", [128, QK_TILES * DT * 128], bf16,
                         kind="ExternalInput")
    wv = nc.dram_tensor("wv", [128, 2 * DT * 512], bf16,
                        kind="ExternalInput")
    wo = nc.dram_tensor("wo", [D, LOCAL], bf16, kind="ExternalInput")
    cost = nc.dram_tensor("cost", [ROT, S], bf16, kind="ExternalInput")
    sint = nc.dram_tensor("sint", [ROT, S], bf16, kind="ExternalInput")
    rt = nc.dram_tensor("rt", [ROT, ROT], bf16, kind="ExternalInput")
    ones = nc.dram_tensor("ones", [128, 1], bf16, kind="ExternalInput")
    onesr = nc.dram_tensor("onesr", [1, 128], bf16, kind="ExternalInput")
    masks = nc.dram_tensor("masks", [128, 4, 512], f32, kind="ExternalInput")
    y = nc.dram_tensor("y", [LOCAL, S], f32, kind="ExternalOutput")

    xt_r = xt.ap().rearrange("p (ss dt s) -> p ss dt s", ss=2, dt=DT)
    wqk_r = wqk.ap().rearrange("p (m dt c) -> p m dt c", m=QK_TILES, dt=DT)
    wv_r = wv.ap().rearrange("p (vn dt c) -> p vn dt c", vn=2, dt=DT)
    wo_r = wo.ap().rearrange("(dt p) c -> p dt c", p=128)

    with tile.TileContext(nc) as tc:
        with tc.tile_pool(name="dram", bufs=1, space="DRAM") as dpool, \
             tc.tile_pool(name="const", bufs=1) as cpool, \
             tc.tile_pool(name="psum", bufs=8, space="PSUM") as psp:
            og_in = dpool.tile([LOCAL, S], bf16, name="og_in")
            og_out = dpool.tile([MP * LOCAL, S], bf16, name="og_out")
            og_in3 = dpool.tile([4 * 256, 512], bf16, name="og_in3")
            og_out3 = dpool.tile([4 * 1024, 512], bf16, name="og_out3")

            rt_sb = cpool.tile([ROT, ROT], bf16, name="rt_sb")
            nc.sync.dma_start(rt_sb[:], rt.ap())
            ones_sb = cpool.tile([128, 1], bf16, name="ones_sb")
            nc.sync.dma_start(ones_sb[:], ones.ap())
            onesr_sb = cpool.tile([1, 128], bf16, name="onesr_sb")
            nc.sync.dma_start(onesr_sb[:], onesr.ap())

            tens = (xt_r, wqk_r, wv_r, wo_r, cost, sint, masks, y,
                    og_in, og_out, og_in3, og_out3, rt_sb, ones_sb,
                    onesr_sb)
            for rep in range(n_repeat):
                _emit_body(nc, tc, tens, psp, cpool, with_collective, rep,
                           phases=phases)

    nc.compile()
    return nc


def _rotary_tables(position_ids):
    """Transposed, interleave-repeated sin/cos tables: [64, S] per batch."""
    import ml_dtypes
    pos = np.asarray(position_ids).astype(np.int64)
    inv_freq = 1.0 / (10000.0 ** (np.arange(0, ROT, 2, dtype=np.float32) / ROT))
    sinusoid = np.arange(2048, dtype=np.float32)[:, None] * inv_freq[None, :]
    sin_t = np.sin(sinusoid).astype(np.float32)   # [2048, 32]
    cos_t = np.cos(sinusoid).astype(np.float32)
    outs = []
    for b in range(pos.shape[0]):
        sg = np.repeat(sin_t[pos[b]], 2, axis=1).T   # [64, S]
        cg = np.repeat(cos_t[pos[b]], 2, axis=1).T
        outs.append((np.ascontiguousarray(sg).astype(ml_dtypes.bfloat16),
                     np.ascontiguousarray(cg).astype(ml_dtypes.bfloat16)))
    return outs


def _consts():
    import ml_dtypes
    rt_np = np.zeros((ROT, ROT), dtype=np.float32)
    for i in range(ROT // 2):
        rt_np[2 * i + 1, 2 * i] = -1.0   # rt = R^T for rotate_every_two
        rt_np[2 * i, 2 * i + 1] = 1.0
    rt_np = rt_np.astype(ml_dtypes.bfloat16)
    ones_np = np.ones((128, 1), dtype=ml_dtypes.bfloat16)
    onesr_np = np.ones((1, 128), dtype=ml_dtypes.bfloat16)
    masks_np = np.zeros((128, 4, 512), dtype=np.float32)
    ii = np.arange(128)[:, None]
    qq = np.arange(512)[None, :]
    for j in range(4):
        masks_np[:, j, :] = (128 * j + ii <= qq).astype(np.float32)
    return rt_np, ones_np, onesr_np, masks_np


def _tile_x(xt):
    # [4096, 2048] -> [128, 2(ss) * 32(dt) * 1024(s)]
    return np.ascontiguousarray(
        xt.reshape(DT, 128, 2, 1024).transpose(1, 2, 0, 3).reshape(128, -1))


def _tile_wqk(w):
    # [4096, 1024] -> [128, 8(m) * 32(dt) * 128(c)]
    return np.ascontiguousarray(
        w.reshape(DT, 128, 8, 128).transpose(1, 2, 0, 3).reshape(128, -1))


def _tile_wv(w):
    # [4096, 1024] -> [128, 2(vn) * 32(dt) * 512(c)]
    return np.ascontiguousarray(
        w.reshape(DT, 128, 2, 512).transpose(1, 2, 0, 3).reshape(128, -1))


def _in_maps(hidden_states, position_ids, W_qkv, W_out):
    import ml_dtypes
    bf16 = ml_dtypes.bfloat16
    hs = np.asarray(hidden_states, dtype=np.float32)
    wqkv = np.asarray(W_qkv, dtype=np.float32)
    wout = np.asarray(W_out, dtype=np.float32)
    rt_np, ones_np, onesr_np, masks_np = _consts()
    trig = _rotary_tables(position_ids)

    xts = [_tile_x(np.ascontiguousarray(hs[b].T).astype(bf16))
           for b in range(B)]
    in_maps = []
    for c in range(N_CORES):
        dp, tp = c // MP, c % MP
        wl = wqkv[:, tp * 3 * LOCAL:(tp + 1) * 3 * LOCAL].astype(bf16)
        sg, cg = trig[dp]
        wq_t = _tile_wqk(wl[:, 0:LOCAL])
        wk_t = _tile_wqk(wl[:, 2 * LOCAL:3 * LOCAL])
        in_maps.append({
            "xt": xts[dp],
            "wqk": np.concatenate([wq_t, wk_t], axis=1),
            "wv": _tile_wv(wl[:, LOCAL:2 * LOCAL]),
            "wo": np.ascontiguousarray(
                wout[:, tp * LOCAL:(tp + 1) * LOCAL]).astype(bf16),
            "cost": cg, "sint": sg,
            "rt": rt_np, "ones": ones_np, "onesr": onesr_np,
            "masks": masks_np,
        })
    return in_maps


def _get_runner(n_repeat=1, phases=(1, 2, 3), with_collective=True):
    key = ("runner", n_repeat, tuple(phases), with_collective)
    if key in _CACHE:
        return _CACHE[key]
    import jax
    from jax.sharding import Mesh, PartitionSpec, NamedSharding
    from jax.experimental.shard_map import shard_map
    from concourse import bass2jax, mybir

    nc = _build_program(with_collective=with_collective, n_repeat=n_repeat,
                        phases=phases)
    bass2jax.install_neuronx_cc_hook()

    partition_name = (nc.partition_id_tensor.name
                      if nc.partition_id_tensor else None)
    in_names, out_names, out_avals, zero_outs = [], [], [], []
    for alloc in nc.m.functions[0].allocations:
        if not isinstance(alloc, mybir.MemoryLocationSet):
            continue
        name = alloc.memorylocations[0].name
        if alloc.kind == "ExternalInput":
            if name != partition_name:
                in_names.append(name)
        elif alloc.kind == "ExternalOutput":
            shape = tuple(alloc.tensor_shape)
            dtype = mybir.dt.np(alloc.dtype)
            out_names.append(name)
            out_avals.append(jax.core.ShapedArray(shape, dtype))
            zero_outs.append(np.zeros(shape, dtype))
    n_params = len(in_names)
    all_names = in_names + out_names
    if partition_name is not None:
        all_names = all_names + [partition_name]

    def _body(*args):
        operands = list(args)
        if partition_name is not None:
            operands.append(bass2jax.partition_id_tensor())
        outs = bass2jax._bass_exec_p.bind(
            *operands,
            out_avals=tuple(out_avals),
            in_names=tuple(all_names),
            out_names=tuple(out_names),
            lowering_input_output_aliases=(),
            sim_require_finite=True,
            sim_require_nnan=True,
            nc=nc,
        )
        return tuple(outs)

    devices = jax.devices()[:N_CORES]
    mesh = Mesh(np.asarray(devices), ("core",))
    n_outs = len(out_names)
    sharded = jax.jit(
        shard_map(_body, mesh=mesh,
                  in_specs=(PartitionSpec("core"),) * (n_params + n_outs),
                  out_specs=(PartitionSpec("core"),) * n_outs,
                  check_rep=False),
        keep_unused=True,
    )
    sharding = NamedSharding(mesh, PartitionSpec("core"))
    runner = {
        "nc": nc, "sharded": sharded, "in_names": in_names,
        "out_names": out_names, "out_avals": out_avals,
        "zero_outs": zero_outs, "sharding": sharding, "jax": jax,
    }
    _CACHE[key] = runner
    return runner


def _stage(runner, in_maps):
    jax = runner["jax"]
    concat_in = [
        np.concatenate([np.asarray(in_maps[c][name]) for c in range(N_CORES)],
                       axis=0)
        for name in runner["in_names"]
    ]
    concat_zero = [
        np.zeros((N_CORES * z.shape[0], *z.shape[1:]), z.dtype)
        for z in runner["zero_outs"]
    ]
    return [jax.device_put(a, runner["sharding"]) for a in concat_in + concat_zero]


def _execute(runner, staged):
    jax = runner["jax"]
    outs = runner["sharded"](*staged)
    outs = jax.block_until_ready(outs)
    return outs


def kernel(hidden_states, position_ids, W_qkv, W_out):
    runner = _get_runner()
    in_maps = _in_maps(hidden_states, position_ids, W_qkv, W_out)
    staged = _stage(runner, in_maps)
    outs = _execute(runner, staged)
    yc = np.asarray(outs[0]).reshape(N_CORES, LOCAL, S)
    result = np.empty((B, S, D), dtype=np.float32)
    for c in range(N_CORES):
        dp, tp = c // MP, c % MP
        result[dp][:, tp * LOCAL:(tp + 1) * LOCAL] = yc[c].T
    return result


def bench(inputs, iters=10, n_repeat=1):
    """Return per-call wall-clock seconds (list) for the staged executable."""
    import time
    runner = _get_runner(n_repeat)
    in_maps = _in_maps(**inputs)
    staged = _stage(runner, in_maps)
    _execute(runner, staged)  # warm-up / compile
    times = []
    for _ in range(iters):
        t0 = time.perf_counter()
        _execute(runner, staged)
        times.append(time.perf_counter() - t0)
    return times
